# revision 1
# baseline (speedup 1.0000x reference)
"""7x7 valid conv2d on [8192, 8192] fp32, distributed over 8 NeuronCores.

Strategy: row-shard the image across 8 cores (host-side overlapping slices
provide the 6-row halo). On each core the convolution runs on the tensor
engine as banded-Toeplitz matmuls: for kernel column j, a stationary matrix
B_j[p, m] = weight[p - m, j] turns a matmul over 128 input rows into a 7-tap
convolution along H producing 122 output rows; the 7 kernel columns
accumulate in PSUM using column-shifted rhs windows. Bias is folded into the
PSUM->SBUF copy as an immediate.
"""

import numpy as np

KH = KW = 7
H = W = 8192
OH = H - KH + 1  # 8186
OW = W - KW + 1
NCORES = 8
P = 128
M_FULL = P - (KH - 1)  # 122 output rows per row-tile
N_TILE = 512

BAND = 1024              # output rows per core (last core overlaps)
B_IN = BAND + KH - 1     # 1030 input rows per core

MM_DTYPE = "float32r"    # matmul operand dtype: float32r | float32
LOOP_ORDER = "nj"        # "nj": col-tile outer, taps inner; "jn": taps outer over psum-bank groups
CHUNK = 8192             # x load chunk width (cols); must be multiple of N_TILE
REPS = 1                 # body repetitions (for slope timing only)
X_BUFS = 2               # chunk-tile buffers
PS_BUFS = 8
O_BUFS = 6

# Isolation switches (timing experiments only — break correctness):
SKIP_OUT_DMA = False     # store only one tile per row-tile
SKIP_X_DMA = False       # load only one chunk per row-tile, reuse for all col-tiles
OUT_ENGINE = "mix"       # sync | gpsimd | scalar | mix (alternate gpsimd/scalar)
OUT_WIDE = False         # accumulate full-width output rows in SBUF, one store per row-tile


def _build_program(bias_val, band_out, w_in):
    import concourse.bacc as bacc
    import concourse.mybir as mybir
    import concourse.tile as tile

    mm_dt = getattr(mybir.dt, MM_DTYPE)
    f32 = mybir.dt.float32
    w_out = w_in - KW + 1

    nc = bacc.Bacc(
        "TRN2",
        target_bir_lowering=False,
        debug=False,
        enable_asserts=False,
        num_devices=NCORES,
    )

    x_dram = nc.dram_tensor("x", [band_out + KH - 1, w_in], mm_dt, kind="ExternalInput")
    wb_dram = nc.dram_tensor("wband", [P, KW, M_FULL], mm_dt, kind="ExternalInput")
    out_dram = nc.dram_tensor("out", [band_out, w_out], f32, kind="ExternalOutput")

    n_row_tiles = (band_out + M_FULL - 1) // M_FULL
    n_col_tiles = (w_out + N_TILE - 1) // N_TILE
    tiles_per_chunk = CHUNK // N_TILE
    chunk_w = CHUNK + KW + 1  # halo so every col-tile's shifted window stays inside

    with tile.TileContext(nc) as tc:
        with (
            tc.tile_pool(name="const", bufs=1) as cpool,
            tc.tile_pool(name="xp", bufs=X_BUFS) as xpool,
            tc.tile_pool(name="op", bufs=O_BUFS) as opool,
            tc.tile_pool(name="pp", bufs=PS_BUFS, space="PSUM") as pspool,
        ):
            w_sb = cpool.tile([P, KW, M_FULL], mm_dt)
            nc.sync.dma_start(w_sb[:], wb_dram.ap()[:])

            def emit_out(t, r0, m, c0, wn, ps):
                o_sb = opool.tile([M_FULL, N_TILE], f32, tag="o", name="o_sb")
                nc.vector.tensor_scalar_add(o_sb[:m, :wn], ps[:m, :wn], bias_val)
                if SKIP_OUT_DMA and c0 != 0:
                    return
                if OUT_ENGINE == "mix":
                    eng = nc.gpsimd if (c0 // N_TILE) % 2 else nc.scalar
                else:
                    eng = getattr(nc, OUT_ENGINE)
                eng.dma_start(
                    out_dram.ap()[r0 : r0 + m, c0 : c0 + wn], o_sb[:m, :wn]
                )

            for _rep in range(REPS):
                for t in range(n_row_tiles):
                    r0 = t * M_FULL
                    m = min(M_FULL, band_out - r0)
                    k = m + KH - 1
                    # chunked x load: chunk g serves col-tiles [g*tpc, (g+1)*tpc)
                    chunks = []
                    for g0 in range(0, w_in, CHUNK):
                        if SKIP_X_DMA and g0 != 0:
                            chunks.append(chunks[0])
                            continue
                        cw = min(chunk_w, w_in - g0)
                        x_sb = xpool.tile([P, chunk_w], mm_dt, tag="x", name="x_sb")
                        nc.sync.dma_start(
                            x_sb[:k, :cw], x_dram.ap()[r0 : r0 + k, g0 : g0 + cw]
                        )
                        chunks.append(x_sb)

                    def rhs(n, j, wn):
                        g = n // tiles_per_chunk
                        loc = (n % tiles_per_chunk) * N_TILE + j
                        return chunks[g][:k, loc : loc + wn]

                    if LOOP_ORDER == "nj":
                        ow = (
                            opool.tile([M_FULL, w_in], f32, tag="ow", bufs=2, name="ow")
                            if OUT_WIDE
                            else None
                        )
                        for n in range(n_col_tiles):
                            c0 = n * N_TILE
                            wn = min(N_TILE, w_out - c0)
                            ps = pspool.tile([M_FULL, N_TILE], f32, tag="ps", name="ps")
                            for j in range(KW):
                                nc.tensor.matmul(
                                    ps[:m, :wn], w_sb[:k, j, :m], rhs(n, j, wn),
                                    start=(j == 0), stop=(j == KW - 1),
                                )
                            if OUT_WIDE:
                                nc.vector.tensor_scalar_add(
                                    ow[:m, c0 : c0 + wn], ps[:m, :wn], bias_val
                                )
                            else:
                                emit_out(t, r0, m, c0, wn, ps)
                        if OUT_WIDE and not SKIP_OUT_DMA:
                            getattr(nc, OUT_ENGINE).dma_start(
                                out_dram.ap()[r0 : r0 + m, :], ow[:m, :w_out]
                            )
                    else:  # "jn": taps outer over groups of PS_BUFS col-tiles
                        for nb in range(0, n_col_tiles, PS_BUFS):
                            group = range(nb, min(nb + PS_BUFS, n_col_tiles))
                            pss = {
                                n: pspool.tile(
                                    [M_FULL, N_TILE], f32, tag="ps", name="ps"
                                )
                                for n in group
                            }
                            for j in range(KW):
                                for n in group:
                                    wn = min(N_TILE, w_out - n * N_TILE)
                                    nc.tensor.matmul(
                                        pss[n][:m, :wn], w_sb[:k, j, :m], rhs(n, j, wn),
                                        start=(j == 0), stop=(j == KW - 1),
                                    )
                            for n in group:
                                wn = min(N_TILE, w_out - n * N_TILE)
                                emit_out(t, r0, m, n * N_TILE, wn, pss[n])

    nc.compile()
    return nc


def _make_wband(weight):
    wband = np.zeros((P, KW, M_FULL), np.float32)
    idx = np.arange(M_FULL)
    for j in range(KW):
        for d in range(KH):
            wband[idx + d, j, idx] = weight[d, j]
    return wband


class Runner:
    """Compiles the per-core program once and exposes repeatable execution
    on all cores via PJRT (the axon path of run_bass_kernel_spmd, inlined so
    inputs can stay device-resident and calls can be timed)."""

    def __init__(self, bias_val, band_out=BAND, w_in=W, n_cores=NCORES):
        self._setup(_build_program(bias_val, band_out, w_in), n_cores)

    @classmethod
    def from_nc(cls, nc, n_cores=NCORES):
        r = cls.__new__(cls)
        r._setup(nc, n_cores)
        return r

    def _setup(self, nc, n_cores):
        import jax
        import concourse.mybir as mybir
        from concourse import bass2jax
        from jax.sharding import Mesh, PartitionSpec
        from jax.experimental.shard_map import shard_map

        self.n_cores = n_cores
        self.nc = nc
        bass2jax.install_neuronx_cc_hook()

        partition_name = (
            nc.partition_id_tensor.name if nc.partition_id_tensor else None
        )
        in_names, out_names, out_avals = [], [], []
        for alloc in nc.m.functions[0].allocations:
            if not isinstance(alloc, mybir.MemoryLocationSet):
                continue
            name = alloc.memorylocations[0].name
            if alloc.kind == "ExternalInput":
                if name != partition_name:
                    in_names.append(name)
            elif alloc.kind == "ExternalOutput":
                out_names.append(name)
                out_avals.append(
                    jax.core.ShapedArray(
                        tuple(alloc.tensor_shape), mybir.dt.np(alloc.dtype)
                    )
                )
        self.in_names, self.out_names, self.out_avals = in_names, out_names, out_avals
        n_params = len(in_names)
        donate = tuple(range(n_params, n_params + len(out_names)))

        def _body(*args):
            operands = list(args)
            if nc.partition_id_tensor is not None:
                operands.append(bass2jax.partition_id_tensor())
            outs = bass2jax._bass_exec_p.bind(
                *operands,
                out_avals=tuple(out_avals),
                in_names=tuple(in_names + out_names)
                + ((nc.partition_id_tensor.name,) if nc.partition_id_tensor else ()),
                out_names=tuple(out_names),
                lowering_input_output_aliases=(),
                sim_require_finite=True,
                sim_require_nnan=True,
                nc=nc,
            )
            return tuple(outs)

        devices = jax.devices()[:n_cores]
        self.mesh = Mesh(np.asarray(devices), ("core",))
        self.pspec = PartitionSpec("core")
        in_specs = (self.pspec,) * (n_params + len(out_names))
        out_specs = (self.pspec,) * len(out_names)
        self.fn = jax.jit(
            shard_map(
                _body,
                mesh=self.mesh,
                in_specs=in_specs,
                out_specs=out_specs,
                check_rep=False,
            ),
            donate_argnums=donate,
            keep_unused=True,
        )

    def put_inputs(self, in_maps):
        """device_put per-core input dicts; returns list of jax arrays."""
        import jax
        from jax.sharding import NamedSharding

        sharding = NamedSharding(self.mesh, self.pspec)
        arrs = []
        for name in self.in_names:
            cat = np.concatenate([np.asarray(m[name]) for m in in_maps], axis=0)
            arrs.append(jax.device_put(cat, sharding))
        return arrs

    def zero_outs(self):
        import jax
        from jax.sharding import NamedSharding

        sharding = NamedSharding(self.mesh, self.pspec)
        return tuple(
            jax.device_put(
                np.zeros((self.n_cores * a.shape[0], *a.shape[1:]), a.dtype), sharding
            )
            for a in self.out_avals
        )

    def run(self, in_arrs, out_bufs):
        """One execution; returns new device output arrays (donates out_bufs)."""
        return self.fn(*in_arrs, *out_bufs)

    def gather(self, outs):
        """Device outputs -> list of per-core dicts of np arrays."""
        res = []
        for c in range(self.n_cores):
            d = {}
            for i, name in enumerate(self.out_names):
                a = self.out_avals[i]
                d[name] = np.asarray(outs[i]).reshape(self.n_cores, *a.shape)[c]
            res.append(d)
        return res


def make_in_maps(x, weight, starts, band_in=B_IN):
    wband = _make_wband(weight)
    return [
        {"x": np.ascontiguousarray(x[s : s + band_in]), "wband": wband}
        for s in starts
    ]


def kernel(x, weight, bias):
    from concourse import bass_utils

    x = np.asarray(x, dtype=np.float32)
    weight = np.asarray(weight, dtype=np.float32)
    bias = np.asarray(bias, dtype=np.float32)

    starts = [min(i * BAND, OH - BAND) for i in range(NCORES)]
    nc = _build_program(float(bias[0]), BAND, W)
    res = bass_utils.run_bass_kernel_spmd(
        nc, make_in_maps(x, weight, starts), core_ids=list(range(NCORES))
    )

    out = np.empty((OH, OW), np.float32)
    for s, r in zip(starts, res.results):
        out[s : s + BAND] = r["out"]
    return out



# revision 5
# speedup vs baseline: 4.2921x; 4.2921x over previous
"""7x7 valid conv2d on [8192, 8192] fp32, distributed over 8 NeuronCores.

Strategy: row-shard the image across 8 cores (1024 output rows each; host-side
overlapping slices provide the 6-row halo). On each core the conv runs as
patch-packed matmuls in bf16: the host packs x into 16x8 pixel patches laid
across the 128 SBUF partitions (partition p = dr*8+dc, free axis = patch
index). Each 16x8 output patch draws on the 2x2 neighborhood of input patches,
so 4 accumulating matmuls with per-alignment stationary matrices B_q[128,128]
produce 128 output pixels per streamed column — 4 PE cycles per 128 outputs vs
the banded-Toeplitz formulation's 7 per 122. Bias is folded into the
PSUM->SBUF copy; the output leaves the device patch-packed bf16 and the host
unpacks/upcasts.
"""

import numpy as np

KH = KW = 7
H = W = 8192
OH = H - KH + 1  # 8186
OW = W - KW + 1
NCORES = 8
P = 128

PR, PC = 16, 8          # patch rows x cols = 128 pixels
BAND = 1024             # output rows per core (last core overlaps)
NA = BAND // PR         # 64 output row-patches per core
NAI = NA + 1            # 65 input row-bands (one halo band)
NB = W // PC            # 1024 col-patches of output (host trims to OW)
NBI = NB + 1            # 1025 input col-patches (one halo patch)
N_TILE = 512            # output patches per PSUM tile (1 bank)

REPS = 1                # body repetitions (for slope timing only)
X_BUFS = 4              # input band buffers (need a, a+1 live + prefetch)
PS_BUFS = 8
O_BUFS = 4

ALIGNS = [(0, 0), (0, 1), (1, 0), (1, 1)]


def _build_program(bias_val):
    import concourse.bacc as bacc
    import concourse.mybir as mybir
    import concourse.tile as tile

    bf16 = mybir.dt.bfloat16
    f32 = mybir.dt.float32

    nc = bacc.Bacc(
        "TRN2",
        target_bir_lowering=False,
        debug=False,
        enable_asserts=False,
        num_devices=NCORES,
    )

    x_dram = nc.dram_tensor("xp", [P, NAI * NBI], bf16, kind="ExternalInput")
    w_dram = nc.dram_tensor("wq", [P, 4, P], bf16, kind="ExternalInput")
    out_dram = nc.dram_tensor("out", [P, NA * NB], bf16, kind="ExternalOutput")

    with tile.TileContext(nc) as tc:
        with (
            tc.tile_pool(name="const", bufs=1) as cpool,
            tc.tile_pool(name="xp", bufs=X_BUFS) as xpool,
            tc.tile_pool(name="op", bufs=O_BUFS) as opool,
            tc.tile_pool(name="pp", bufs=PS_BUFS, space="PSUM") as pspool,
        ):
            w_sb = cpool.tile([P, 4, P], bf16)
            nc.sync.dma_start(w_sb[:], w_dram.ap()[:])
            bias_sb = cpool.tile([P, 1], f32)
            nc.vector.memset(bias_sb[:], bias_val)

            for _rep in range(REPS):
                def load_band(a):
                    t = xpool.tile([P, NBI], bf16, tag="x", name="x_sb")
                    nc.sync.dma_start(
                        t[:], x_dram.ap()[:, a * NBI : (a + 1) * NBI]
                    )
                    return t

                bands = {0: load_band(0), 1: load_band(1)}
                for a in range(NA):
                    if a + 2 < NAI:
                        bands[a + 2] = load_band(a + 2)
                    o_sb = opool.tile([P, NB], bf16, tag="o", name="o_sb")
                    for ti, b0 in enumerate(range(0, NB, N_TILE)):
                        ps = pspool.tile([P, N_TILE], f32, tag="ps", name="ps")
                        for q, (qa, qb) in enumerate(ALIGNS):
                            nc.tensor.matmul(
                                ps[:],
                                w_sb[:, q, :],
                                bands[a + qa][:, b0 + qb : b0 + qb + N_TILE],
                                start=(q == 0),
                                stop=(q == 3),
                            )
                        if ti % 2 == 0:
                            nc.scalar.activation(
                                o_sb[:, b0 : b0 + N_TILE], ps[:],
                                mybir.ActivationFunctionType.Identity,
                                bias=bias_sb[:],
                            )
                        else:
                            nc.vector.tensor_scalar_add(
                                o_sb[:, b0 : b0 + N_TILE], ps[:], bias_val
                            )
                    bands.pop(a, None)
                    nc.gpsimd.dma_start(
                        out_dram.ap()[:, a * NB : (a + 1) * NB], o_sb[:]
                    )

    nc.compile()
    return nc


def _bf16():
    import ml_dtypes

    return ml_dtypes.bfloat16


def _make_wq(weight):
    """Stationary matrices B_q[p, m]: p = dr*8+dc input pixel of patch
    (a+qa, b+qb), m = or*8+oc output pixel of patch (a, b). Returned in
    DRAM layout [p, q, m] bf16."""
    wq = np.zeros((4, P, P), np.float32)
    dr, dc = np.arange(PR)[:, None], np.arange(PC)[None, :]
    orr, occ = np.arange(PR)[:, None], np.arange(PC)[None, :]
    for q, (qa, qb) in enumerate(ALIGNS):
        # dh = PR*qa + dr - or, dw = PC*qb + dc - oc; valid in [0, 6]
        dh = PR * qa + dr.reshape(-1, 1, 1, 1) - orr.reshape(1, 1, -1, 1)
        dw = PC * qb + dc.reshape(1, -1, 1, 1) - occ.reshape(1, 1, 1, -1)
        valid = (dh >= 0) & (dh < KH) & (dw >= 0) & (dw < KW)
        vals = np.where(valid, weight[np.clip(dh, 0, KH - 1), np.clip(dw, 0, KW - 1)], 0.0)
        wq[q] = vals.reshape(P, P)
    return np.ascontiguousarray(wq.transpose(1, 0, 2)).astype(_bf16())


def _pack_x(x16, s):
    """Pack band starting at row s into [128, NAI*NBI] bf16.
    x16: full [H, W] bf16 array."""
    bf16 = _bf16()
    rows = PR * NAI
    xb = np.zeros((rows, PC * NBI), bf16)
    avail = min(rows, H - s)
    xb[:avail, :W] = x16[s : s + avail]
    v = xb.reshape(NAI, PR, NBI, PC)
    return np.ascontiguousarray(v.transpose(1, 3, 0, 2)).reshape(P, NAI * NBI)


def unpack_out(o_packed):
    """[128, NA*NB] bf16 -> [BAND, OW] fp32 band."""
    v = np.asarray(o_packed).reshape(PR, PC, NA, NB)
    band = v.transpose(2, 0, 3, 1).reshape(BAND, NB * PC)
    return band[:, :OW].astype(np.float32)


def make_in_maps(x, weight, starts):
    x16 = np.asarray(x, np.float32).astype(_bf16())
    wq = _make_wq(np.asarray(weight, np.float32))
    return [{"xp": _pack_x(x16, s), "wq": wq} for s in starts]


class Runner:
    """Compiles the per-core program once and exposes repeatable execution
    on all cores via PJRT (the axon path of run_bass_kernel_spmd, inlined so
    inputs can stay device-resident and calls can be timed)."""

    def __init__(self, bias_val):
        self._setup(_build_program(bias_val), NCORES)

    @classmethod
    def from_nc(cls, nc, n_cores=NCORES):
        r = cls.__new__(cls)
        r._setup(nc, n_cores)
        return r

    def _setup(self, nc, n_cores):
        import jax
        import concourse.mybir as mybir
        from concourse import bass2jax
        from jax.sharding import Mesh, PartitionSpec
        from jax.experimental.shard_map import shard_map

        self.n_cores = n_cores
        self.nc = nc
        bass2jax.install_neuronx_cc_hook()

        partition_name = (
            nc.partition_id_tensor.name if nc.partition_id_tensor else None
        )
        in_names, out_names, out_avals = [], [], []
        for alloc in nc.m.functions[0].allocations:
            if not isinstance(alloc, mybir.MemoryLocationSet):
                continue
            name = alloc.memorylocations[0].name
            if alloc.kind == "ExternalInput":
                if name != partition_name:
                    in_names.append(name)
            elif alloc.kind == "ExternalOutput":
                out_names.append(name)
                out_avals.append(
                    jax.core.ShapedArray(
                        tuple(alloc.tensor_shape), mybir.dt.np(alloc.dtype)
                    )
                )
        self.in_names, self.out_names, self.out_avals = in_names, out_names, out_avals
        n_params = len(in_names)
        donate = tuple(range(n_params, n_params + len(out_names)))

        def _body(*args):
            operands = list(args)
            if nc.partition_id_tensor is not None:
                operands.append(bass2jax.partition_id_tensor())
            outs = bass2jax._bass_exec_p.bind(
                *operands,
                out_avals=tuple(out_avals),
                in_names=tuple(in_names + out_names)
                + ((nc.partition_id_tensor.name,) if nc.partition_id_tensor else ()),
                out_names=tuple(out_names),
                lowering_input_output_aliases=(),
                sim_require_finite=True,
                sim_require_nnan=True,
                nc=nc,
            )
            return tuple(outs)

        devices = jax.devices()[:n_cores]
        self.mesh = Mesh(np.asarray(devices), ("core",))
        self.pspec = PartitionSpec("core")
        in_specs = (self.pspec,) * (n_params + len(out_names))
        out_specs = (self.pspec,) * len(out_names)
        self.fn = jax.jit(
            shard_map(
                _body,
                mesh=self.mesh,
                in_specs=in_specs,
                out_specs=out_specs,
                check_rep=False,
            ),
            donate_argnums=donate,
            keep_unused=True,
        )

    def put_inputs(self, in_maps):
        """device_put per-core input dicts; returns list of jax arrays."""
        import jax
        from jax.sharding import NamedSharding

        sharding = NamedSharding(self.mesh, self.pspec)
        arrs = []
        for name in self.in_names:
            cat = np.concatenate([np.asarray(m[name]) for m in in_maps], axis=0)
            arrs.append(jax.device_put(cat, sharding))
        return arrs

    def zero_outs(self):
        import jax
        from jax.sharding import NamedSharding

        sharding = NamedSharding(self.mesh, self.pspec)
        return tuple(
            jax.device_put(
                np.zeros((self.n_cores * a.shape[0], *a.shape[1:]), a.dtype), sharding
            )
            for a in self.out_avals
        )

    def run(self, in_arrs, out_bufs):
        """One execution; returns new device output arrays (donates out_bufs)."""
        return self.fn(*in_arrs, *out_bufs)

    def gather(self, outs):
        """Device outputs -> list of per-core dicts of np arrays."""
        res = []
        for c in range(self.n_cores):
            d = {}
            for i, name in enumerate(self.out_names):
                a = self.out_avals[i]
                d[name] = np.asarray(outs[i]).reshape(self.n_cores, *a.shape)[c]
            res.append(d)
        return res


def kernel(x, weight, bias):
    from concourse import bass_utils

    x = np.asarray(x, dtype=np.float32)
    weight = np.asarray(weight, dtype=np.float32)
    bias = np.asarray(bias, dtype=np.float32)

    starts = [min(i * BAND, OH - BAND) for i in range(NCORES)]
    nc = _build_program(float(bias[0]))
    res = bass_utils.run_bass_kernel_spmd(
        nc, make_in_maps(x, weight, starts), core_ids=list(range(NCORES))
    )

    out = np.empty((OH, OW), np.float32)
    for s, r in zip(starts, res.results):
        out[s : s + BAND] = unpack_out(r["out"])
    return out
